# revision 1
# baseline (speedup 1.0000x reference)
"""Trainium2 Bass kernel for BinaryNormalizedLinear.

Computes (data-parallel over the token dim across 8 NeuronCores):
    W_q = (W > mean(W)).astype(f32)          # global mean over all of W
    b_q = (b > mean(b)).astype(f32)
    z   = x @ W_q.T + b_q                    # [M, OUT]
    out = (z - mean(z, -1)) / (sqrt(var(z, -1, ddof=1)) + 1e-8)

Sharding: x is split along M (rows) across cores; normalization is row-local
so no output collective is needed.  The global W mean is computed by giving
each core 1/8 of W to partial-sum, followed by a tiny AllReduce.
The matmul runs in bf16: W_q is exactly representable ({0,1}), so the only
rounding is bf16(x), giving ~1e-4 relative-to-absmax output error.
"""

import os
from contextlib import ExitStack

import numpy as np

P = 128
N_FREE = 512
EPS = 1e-8


class Cfg:
    def __init__(self, n_cores, M, IN, OUT):
        self.n_cores = n_cores
        self.M = M
        self.IN = IN
        self.OUT = OUT
        self.M_LOC = M // n_cores        # rows of x per core
        self.MB = self.M_LOC // P        # m blocks per core
        self.IB = IN // P                # contraction blocks
        self.OT = OUT // N_FREE          # output column tiles
        self.WSL_F = (OUT * IN) // n_cores // P  # free size of per-core W slice


FULL = Cfg(8, 8192, 4096, 4096)


def emit(ctx, tc, cfg, xT, Wt, wsl, b_in, out_t, pfx=""):
    """Emit the kernel body into TileContext tc.

    xT:   [IB, P, M_LOC] f32   per-core x^T, i on partitions
    Wt:   [OT, IB, P, N_FREE] f32  W^T tiled, i on partitions, o on free
    wsl:  [P, WSL_F] f32       per-core slice of W (for the global mean)
    b_in: [OUT] f32
    out_t:[MB, P, OUT] f32     per-core output rows (m = mb*128 + p)
    """
    import concourse.bass as bass
    import concourse.mybir as mybir
    from concourse import bass_isa

    nc = tc.nc
    f32 = mybir.dt.float32
    bf16 = mybir.dt.bfloat16
    Alu = mybir.AluOpType

    singles = ctx.enter_context(tc.tile_pool(name=pfx + "singles", bufs=1))
    stage = ctx.enter_context(tc.tile_pool(name=pfx + "stage", bufs=2))
    xstage = ctx.enter_context(tc.tile_pool(name=pfx + "xstage", bufs=2))
    wstage = ctx.enter_context(tc.tile_pool(name=pfx + "wstage", bufs=4))
    wqpool = ctx.enter_context(tc.tile_pool(name=pfx + "wqpool", bufs=5))
    wq7pool = ctx.enter_context(tc.tile_pool(name=pfx + "wq7", bufs=cfg.IB // 2))
    ostage = ctx.enter_context(tc.tile_pool(name=pfx + "ostage", bufs=2))
    zio = ctx.enter_context(tc.tile_pool(name=pfx + "zio", bufs=4))
    zrd = ctx.enter_context(tc.tile_pool(name=pfx + "zrd", bufs=2))
    small = ctx.enter_context(tc.tile_pool(name=pfx + "small", bufs=4))
    psum_pool = ctx.enter_context(tc.tile_pool(name=pfx + "psum", bufs=8, space="PSUM"))
    dram = ctx.enter_context(tc.tile_pool(name=pfx + "dram", bufs=1, space="DRAM"))

    # persistent SBUF tensors (split per block so Tile deps stay fine-grained).
    # z for o-tiles 0..OT-2 round-trips through DRAM; the last o-tile is
    # normalized straight from PSUM and never stored.
    x_sb = [
        singles.tile([P, cfg.M_LOC], bf16, tag=f"x{ib}", name=f"{pfx}x{ib}")
        for ib in range(cfg.IB)
    ]
    stats_mb = [
        singles.tile([P, cfg.OT, 6], f32, tag=f"stats{mb}", name=f"{pfx}stats{mb}")
        for mb in range(cfg.MB)
    ]
    bq_sb = singles.tile([P, cfg.OUT], f32, tag="bq_sb")
    mu_bcast = singles.tile([P, 1], f32, tag="mu_bcast")
    Z_W = (cfg.OT - 1) * N_FREE
    z_dram = dram.tile([cfg.MB, P, Z_W], bf16)

    # ---- global mean of W: per-core partial sum + AllReduce ----
    # wsl loads go on the DVE/ACT DMA queues so the sync queue is free to
    # prefetch GEMM W tiles from t=0.
    # staged through the (tail-only) ostage slots: 16KB each, zero extra SBUF
    CH = min(4096, cfg.WSL_F)  # chunk of the W slice reduced per op
    nch = cfg.WSL_F // CH
    wm_parts = singles.tile([P, nch], f32, tag="wm_parts")
    for j in range(nch):
        wm_st = ostage.tile([P, CH], f32, tag="o_t", name=pfx + "wm_st")
        eng = [nc.gpsimd, nc.scalar, nc.sync][j % 3]
        eng.dma_start(wm_st, wsl[:, j * CH : (j + 1) * CH])
        nc.vector.tensor_reduce(
            wm_parts[:, j : j + 1], wm_st, axis=mybir.AxisListType.X, op=Alu.add
        )
    wm_red = small.tile([P, 1], f32, tag="wm_red")
    nc.vector.tensor_reduce(
        wm_red, wm_parts, axis=mybir.AxisListType.X, op=Alu.add
    )
    wsum_bc = small.tile([P, 1], f32, tag="wsum_bc")
    nc.gpsimd.partition_all_reduce(wsum_bc, wm_red, channels=P, reduce_op=bass_isa.ReduceOp.add)

    bounce_in = dram.tile([P, 1], f32)
    bounce_out = dram.tile([P, 1], f32)
    nc.gpsimd.dma_start(bounce_in[:], wsum_bc)
    if cfg.n_cores > 1:
        nc.gpsimd.collective_compute(
            "AllReduce",
            Alu.add,
            replica_groups=[list(range(cfg.n_cores))],
            ins=[bounce_in.opt()],
            outs=[bounce_out.opt()],
        )
    else:
        nc.gpsimd.dma_start(bounce_out[:], bounce_in[:])
    mu_raw = small.tile([P, 1], f32, tag="mu_raw")
    nc.gpsimd.dma_start(mu_raw, bounce_out[:])
    nc.scalar.mul(mu_bcast, mu_raw, 1.0 / float(cfg.OUT * cfg.IN))

    # ---- quantize b and broadcast across partitions (emitted later, just
    # ---- before the first drains, to keep head DMA bandwidth for mu/x/W) ----
    def emit_b_path():
        BF = cfg.OUT // P
        b_pt = singles.tile([P, BF], f32, tag="b_pt", name=pfx + "b_pt")
        nc.scalar.dma_start(b_pt, b_in.rearrange("(p f) -> p f", p=P))
        bsum = small.tile([P, 1], f32, tag="bsum", name=pfx + "bsum")
        nc.vector.tensor_reduce(bsum, b_pt, axis=mybir.AxisListType.X, op=Alu.add)
        bsum_bc = small.tile([P, 1], f32, tag="bsum_bc", name=pfx + "bsum_bc")
        nc.gpsimd.partition_all_reduce(
            bsum_bc, bsum, channels=P, reduce_op=bass_isa.ReduceOp.add
        )
        bmean = small.tile([P, 1], f32, tag="bmean", name=pfx + "bmean")
        nc.scalar.mul(bmean, bsum_bc, 1.0 / float(cfg.OUT))
        bq_pt = singles.tile([P, BF], f32, tag="bq_pt", name=pfx + "bq_pt")
        nc.vector.tensor_scalar(bq_pt, b_pt, bmean, None, op0=Alu.is_gt)
        bq_dram = dram.tile([cfg.OUT], f32, name=pfx + "bq_dram")
        nc.scalar.dma_start(bq_dram.rearrange("(p f) -> p f", p=P), bq_pt)
        nc.scalar.dma_start(bq_sb, bq_dram[None, :].to_broadcast([P, cfg.OUT]))

    def load_x(ib):
        # x loads alternate between the SWDGE and ACT queues, interleaved
        # just-in-time with the first o-tile's W groups, so no single queue
        # backs up during the bandwidth-saturated head.
        x_st = xstage.tile([P, cfg.M_LOC], f32, tag="x_st", name=pfx + "x_st")
        eng = nc.gpsimd if ib % 2 == 0 else nc.scalar
        eng.dma_start(x_st, xT[ib])
        nc.any.tensor_copy(x_sb[ib], x_st)

    def drain_mb(ot, mb, psum):
        # z = psum + b_q (bf16), partial row stats, then park z in DRAM
        osl = slice(ot * N_FREE, (ot + 1) * N_FREE)
        z_t = zio.tile([P, N_FREE], bf16, tag="zio", name=f"{pfx}zw{ot}_{mb}")
        nc.vector.tensor_tensor(z_t, psum, bq_sb[:, osl], op=Alu.add)
        nc.vector.bn_stats(stats_mb[mb][:, ot, :], z_t)
        nc.sync.dma_start(z_dram[mb, :, osl], z_t)

    ddof_scale = float(cfg.OUT) / float(cfg.OUT - 1)

    def normalize_mb(mb, psum7):
        # (z - mean) / (sqrt(var * n/(n-1)) + eps); last o-slice from PSUM.
        # One z-read DMA and one 16KB/partition out DMA per m-block.
        mv = small.tile([P, 2], f32, tag="mv", name=f"{pfx}mv{mb}")
        nc.vector.bn_aggr(mv, stats_mb[mb])
        std = small.tile([P, 1], f32, tag="std", name=f"{pfx}std{mb}")
        nc.scalar.activation(
            std, mv[:, 1:2], mybir.ActivationFunctionType.Sqrt, scale=ddof_scale
        )
        nc.vector.tensor_scalar_add(std, std, EPS)
        rstd = small.tile([P, 1], f32, tag="rstd", name=f"{pfx}rstd{mb}")
        nc.vector.reciprocal(rstd, std)
        z_rt = zrd.tile(
            [P, cfg.OT - 1, N_FREE], bf16, tag="zrd", name=f"{pfx}zrd{mb}"
        )
        nc.sync.dma_start(z_rt, z_dram[mb])
        row_t = ostage.tile(
            [P, cfg.OT, N_FREE], f32, tag="o_t", name=f"{pfx}o{mb}"
        )
        for ot in range(cfg.OT):
            src = psum7 if ot == cfg.OT - 1 else z_rt[:, ot, :]
            nc.vector.tensor_scalar(
                row_t[:, ot, :],
                src,
                mv[:, 0:1],
                rstd,
                op0=Alu.subtract,
                op1=Alu.mult,
            )
        nc.scalar.dma_start(out_t[mb], row_t)

    def load_wq(ot, ibp, pool, tag):
        # one DMA + one binarize per PAIR of i-blocks (halves SP-queue load)
        w_st = wstage.tile(
            [P, 2, N_FREE], f32, tag="w_st", name=f"{pfx}w{ot}_{ibp}"
        )
        nc.sync.dma_start(
            w_st, Wt[ot, 2 * ibp : 2 * ibp + 2].rearrange("b p f -> p b f")
        )
        wq = pool.tile([P, 2, N_FREE], bf16, tag=tag, name=f"{pfx}{tag}{ot}_{ibp}")
        nc.vector.tensor_scalar(wq, w_st, mu_bcast, None, op0=Alu.is_gt)
        return wq

    # ---- main GEMM ----
    # o-tiles 0..OT-2: ib in groups of IBG, all MB m-blocks accumulate in
    # parallel PSUM banks; drains at the end of the o-tile overlap the next
    # o-tile's matmuls.
    IBG = min(8, cfg.IB)
    NG = cfg.IB // IBG
    for ot in range(cfg.OT - 1):
        psums = [
            psum_pool.tile([P, N_FREE], f32, tag="ps", name=f"{pfx}ps{ot}_{mb}")
            for mb in range(cfg.MB)
        ]
        for ig in range(NG):
            wqs = [
                load_wq(ot, (ig * IBG) // 2 + k, wqpool, "wq")
                for k in range(IBG // 2)
            ]
            if ot == 0:
                for i2 in range(IBG):
                    load_x(ig * IBG + i2)
            for mb in range(cfg.MB):
                for i2 in range(IBG):
                    ib = ig * IBG + i2
                    nc.tensor.matmul(
                        psums[mb],
                        lhsT=x_sb[ib][:, mb * P : (mb + 1) * P],
                        rhs=wqs[i2 // 2][:, i2 % 2, :],
                        start=(ib == 0),
                        stop=(ib == cfg.IB - 1),
                    )
        if ot == 0:
            emit_b_path()
        for mb in range(cfg.MB):
            drain_mb(ot, mb, psums[mb])

    # Last o-tile: m-block-outer with this o-tile's W resident, so each
    # m-block's row completes early and its normalize/store overlaps the
    # remaining m-blocks' matmuls.
    ot = cfg.OT - 1
    osl7 = slice(ot * N_FREE, (ot + 1) * N_FREE)
    wq7 = [load_wq(ot, k, wq7pool, "wq7") for k in range(cfg.IB // 2)]
    for mb in range(cfg.MB):
        psum = psum_pool.tile([P, N_FREE], f32, tag="ps", name=f"{pfx}ps{ot}_{mb}")
        for ib in range(cfg.IB):
            nc.tensor.matmul(
                psum,
                lhsT=x_sb[ib][:, mb * P : (mb + 1) * P],
                rhs=wq7[ib // 2][:, ib % 2, :],
                start=(ib == 0),
                stop=(ib == cfg.IB - 1),
            )
        # add b_q in place, then stats straight from PSUM
        nc.vector.tensor_tensor(psum, psum, bq_sb[:, osl7], op=Alu.add)
        nc.vector.bn_stats(stats_mb[mb][:, ot, :], psum)
        normalize_mb(mb, psum)


def build(cfg):
    import concourse.mybir as mybir
    import concourse.tile as tile
    from concourse import bacc

    f32 = mybir.dt.float32
    nc = bacc.Bacc(
        "TRN2",
        target_bir_lowering=False,
        debug=False,
        num_devices=cfg.n_cores,
    )
    xT = nc.dram_tensor("xT", [cfg.IB, P, cfg.M_LOC], f32, kind="ExternalInput").ap()
    Wt = nc.dram_tensor("Wt", [cfg.OT, cfg.IB, P, N_FREE], f32, kind="ExternalInput").ap()
    wsl = nc.dram_tensor("wsl", [P, cfg.WSL_F], f32, kind="ExternalInput").ap()
    b_in = nc.dram_tensor("b_in", [cfg.OUT], f32, kind="ExternalInput").ap()
    out_t = nc.dram_tensor("out", [cfg.MB, P, cfg.OUT], f32, kind="ExternalOutput").ap()

    with tile.TileContext(nc) as tc:
        with ExitStack() as ctx:
            emit(ctx, tc, cfg, xT, Wt, wsl, b_in, out_t)
    nc.compile()
    return nc


def build_repeat(cfg, reps):
    """Build a variant executing the whole kernel `reps` times back-to-back
    (same I/O tensors), for slope-based device timing."""
    import concourse.mybir as mybir
    import concourse.tile as tile
    from concourse import bacc

    f32 = mybir.dt.float32
    nc = bacc.Bacc(
        "TRN2",
        target_bir_lowering=False,
        debug=False,
        num_devices=cfg.n_cores,
    )
    xT = nc.dram_tensor("xT", [cfg.IB, P, cfg.M_LOC], f32, kind="ExternalInput").ap()
    Wt = nc.dram_tensor("Wt", [cfg.OT, cfg.IB, P, N_FREE], f32, kind="ExternalInput").ap()
    wsl = nc.dram_tensor("wsl", [P, cfg.WSL_F], f32, kind="ExternalInput").ap()
    b_in = nc.dram_tensor("b_in", [cfg.OUT], f32, kind="ExternalInput").ap()
    out_t = nc.dram_tensor("out", [cfg.MB, P, cfg.OUT], f32, kind="ExternalOutput").ap()

    with tile.TileContext(nc) as tc:
        for r in range(reps):
            with ExitStack() as ctx:
                emit(ctx, tc, cfg, xT, Wt, wsl, b_in, out_t, pfx=f"r{r}_")
    nc.compile()
    return nc


def prep_in_maps(x, W, b, cfg):
    x = np.ascontiguousarray(x, dtype=np.float32)
    W = np.ascontiguousarray(W, dtype=np.float32)
    b = np.ascontiguousarray(b, dtype=np.float32)
    Wt = np.ascontiguousarray(
        W.reshape(cfg.OT, N_FREE, cfg.IB, P).transpose(0, 2, 3, 1)
    )
    rows_per_core = cfg.OUT // cfg.n_cores
    in_maps = []
    for c in range(cfg.n_cores):
        xc = x[c * cfg.M_LOC : (c + 1) * cfg.M_LOC]
        xT = np.ascontiguousarray(xc.reshape(cfg.M_LOC, cfg.IB, P).transpose(1, 2, 0))
        wsl = np.ascontiguousarray(
            W[c * rows_per_core : (c + 1) * rows_per_core].reshape(P, cfg.WSL_F)
        )
        in_maps.append({"xT": xT, "Wt": Wt, "wsl": wsl, "b_in": b})
    return in_maps


class Runner:
    """Executes a compiled Bass module over 8 cores via PJRT (axon), with
    input staging separated from execution so repeated runs can be timed."""

    def __init__(self, nc, n_cores):
        import jax
        import concourse.mybir as mybir
        from concourse.bass2jax import (
            _bass_exec_p,
            install_neuronx_cc_hook,
            partition_id_tensor,
        )
        from jax.experimental.shard_map import shard_map
        from jax.sharding import Mesh, NamedSharding, PartitionSpec

        install_neuronx_cc_hook()
        self.jax = jax
        self.n_cores = n_cores
        partition_name = (
            nc.partition_id_tensor.name if nc.partition_id_tensor else None
        )
        in_names = []
        out_names = []
        out_avals = []
        self.out_shapes = []
        zero_shapes = []
        for alloc in nc.m.functions[0].allocations:
            if not isinstance(alloc, mybir.MemoryLocationSet):
                continue
            name = alloc.memorylocations[0].name
            if alloc.kind == "ExternalInput":
                if name != partition_name:
                    in_names.append(name)
            elif alloc.kind == "ExternalOutput":
                shape = tuple(alloc.tensor_shape)
                dtype = mybir.dt.np(alloc.dtype)
                out_names.append(name)
                out_avals.append(jax.core.ShapedArray(shape, dtype))
                self.out_shapes.append((shape, dtype))
                zero_shapes.append((shape, dtype))
        self.n_params = len(in_names)
        self.in_names = list(in_names)
        self.out_names = list(out_names)
        in_names_full = in_names + out_names
        if partition_name is not None:
            in_names_full.append(partition_name)

        self._p = _bass_exec_p
        self._partition_id_tensor = partition_id_tensor
        self._partition_name = partition_name
        self._bind_kwargs = dict(
            out_avals=tuple(out_avals),
            in_names=tuple(in_names_full),
            out_names=tuple(out_names),
            lowering_input_output_aliases=(),
            sim_require_finite=True,
            sim_require_nnan=True,
            nc=nc,
        )

        def _body(*args):
            operands = list(args)
            if partition_name is not None:
                operands.append(partition_id_tensor())
            outs = _bass_exec_p.bind(*operands, **self._bind_kwargs)
            return tuple(outs)

        devices = jax.devices()[:n_cores]
        assert len(devices) == n_cores
        self.mesh = Mesh(np.asarray(devices), ("core",))
        n_outs = len(out_names)
        in_specs = (PartitionSpec("core"),) * (self.n_params + n_outs)
        out_specs = (PartitionSpec("core"),) * n_outs
        self.fn = jax.jit(
            shard_map(
                _body,
                mesh=self.mesh,
                in_specs=in_specs,
                out_specs=out_specs,
                check_rep=False,
            ),
            keep_unused=True,
        )
        self.sharding = NamedSharding(self.mesh, PartitionSpec("core"))
        self.staged = None

    def stage(self, in_maps):
        jax = self.jax
        concat = [
            np.concatenate([np.asarray(m[name]) for m in in_maps], axis=0)
            for name in self.in_names
        ]
        concat += [
            np.zeros((self.n_cores * s[0], *s[1:]), d) for s, d in self.out_shapes
        ]
        self.staged = [jax.device_put(a, self.sharding) for a in concat]
        self.jax.block_until_ready(self.staged)

    def run(self):
        outs = self.fn(*self.staged)
        self.jax.block_until_ready(outs)
        return {
            name: np.asarray(outs[i]).reshape(
                self.n_cores, *self.out_shapes[i][0]
            )
            for i, name in enumerate(self.out_names)
        }

    def timeit(self, iters=20):
        import time

        ts = []
        for _ in range(iters):
            t0 = time.perf_counter()
            outs = self.fn(*self.staged)
            self.jax.block_until_ready(outs)
            ts.append(time.perf_counter() - t0)
        return ts

    def make_chained(self, k):
        """A jitted fn running the NEFF k times back-to-back in one
        dispatch, serialized by threading the output buffers through."""
        import jax
        from jax.experimental.shard_map import shard_map
        from jax.sharding import PartitionSpec

        n_params = self.n_params
        n_outs = len(self.out_names)

        def _chain(*args):
            params = list(args[:n_params])
            outs = list(args[n_params:])
            for _ in range(k):
                operands = params + outs
                if self._partition_name is not None:
                    operands.append(self._partition_id_tensor())
                outs = list(self._p.bind(*operands, **self._bind_kwargs))
            return tuple(outs)

        in_specs = (PartitionSpec("core"),) * (n_params + n_outs)
        out_specs = (PartitionSpec("core"),) * n_outs
        fn = jax.jit(
            shard_map(
                _chain,
                mesh=self.mesh,
                in_specs=in_specs,
                out_specs=out_specs,
                check_rep=False,
            ),
            keep_unused=True,
        )
        return fn

    def time_chained(self, ks=(1, 9, 17), iters=8):
        """Estimate per-NEFF execution time by regressing wall time on k."""
        import time

        mins = {}
        for k in ks:
            fn = self.make_chained(k)
            outs = fn(*self.staged)
            self.jax.block_until_ready(outs)  # compile + warm
            ts = []
            for _ in range(iters):
                t0 = time.perf_counter()
                outs = fn(*self.staged)
                self.jax.block_until_ready(outs)
                ts.append(time.perf_counter() - t0)
            mins[k] = min(ts)
        ks_l = sorted(mins)
        xs = np.array(ks_l, dtype=np.float64)
        ys = np.array([mins[k] for k in ks_l], dtype=np.float64)
        slope = np.polyfit(xs, ys, 1)[0]
        return slope, mins


_cache = {}


def get_runner():
    cfg = FULL
    if "runner" not in _cache:
        _cache["runner"] = Runner(build(cfg), cfg.n_cores)
    return _cache["runner"]


def kernel(x, W, b):
    cfg = FULL
    assert x.shape == (cfg.M, cfg.IN) and W.shape == (cfg.OUT, cfg.IN)
    r = get_runner()
    r.stage(prep_in_maps(x, W, b, cfg))
    outs = r.run()
    out = outs["out"].reshape(cfg.n_cores * cfg.MB, P, cfg.OUT).reshape(
        cfg.M, cfg.OUT
    )
    return np.ascontiguousarray(out, dtype=np.float32)


kernel.last_exec_ns = None



# revision 2
# speedup vs baseline: 5.1566x; 5.1566x over previous
"""Trainium2 Bass kernel for BinaryNormalizedLinear (v2).

Computes (data-parallel over the token dim across 8 NeuronCores):
    W_q = (W > mean(W)).astype(f32)          # global mean over all of W
    b_q = (b > mean(b)).astype(f32)
    z   = x @ W_q.T + b_q                    # [M, OUT]
    out = (z - mean(z, -1)) / (sqrt(var(z, -1, ddof=1)) + 1e-8)

v2 changes vs baseline:
  - wsl (W-mean input) loads first on the SP queue in 1MB chunks reduced as
    they arrive; W GEMM loads queue behind them -> shorter mean/AllReduce
    critical path.
  - x loaded via gpsimd cast-DMA (f32->bf16 in flight), no DVE copy.
  - 4+4 PSUM bank pipelining per o-tile (two mb-groups) so matmuls never
    wait on PSUM drains; last o-tile in 2-bank groups so rows normalize
    progressively.
  - z parked in DRAM as bf16 in 256KB pair-writes; last two o-tiles stay in
    SBUF (no round trip in the tail).
  - fewer, bigger, per-partition-contiguous DMAs spread across SP/ACT/Pool.
"""

import os
from contextlib import ExitStack

import numpy as np

P = 128
NF = 512
EPS = 1e-8


class Cfg:
    def __init__(self, n_cores, M, IN, OUT):
        self.n_cores = n_cores
        self.M = M
        self.IN = IN
        self.OUT = OUT
        self.M_LOC = M // n_cores        # rows of x per core
        self.MB = self.M_LOC // P        # m blocks per core
        self.IB = IN // P                # contraction blocks
        self.OT = OUT // NF              # output column tiles
        self.WSL_F = (OUT * IN) // n_cores // P  # free size of per-core W slice


FULL = Cfg(8, 8192, 4096, 4096)


def emit(ctx, tc, cfg, xT, Wt, wsl, b_in, out_t, pfx="", flags=frozenset()):
    """Emit the kernel body into TileContext tc.

    flags (timing-probe variants; outputs wrong when non-empty):
      "nocoll"  AllReduce -> local DRAM copy
      "nonorm"  skip z DMA round-trip + stats + normalize/out writes
      "mmhalf"  half the contraction blocks (1024 MMs instead of 2048)
      "nowdma"  one W DMA per o-tile reused for all 8 wq tiles (64->8MB)

    xT:   [IB//2, P, 2*M_LOC] f32   x^T tiles, k on partitions, 2 k-blocks
                                    per tile ([i2*M_LOC + m] on free)
    Wt:   [OT, IB//4, P, 4*NF] f32  W^T tiles, k on partitions, 4 k-blocks
                                    of one o-tile per tile ([j*NF + c])
    wsl:  [P, WSL_F] f32            per-core slice of W (for the global mean)
    b_in: [OUT] f32
    out_t:[MB, P, OUT] f32          per-core output rows (m = mb*128 + p)
    """
    import concourse.bass as bass
    import concourse.mybir as mybir
    from concourse import bass_isa

    nc = tc.nc
    f32 = mybir.dt.float32
    bf16 = mybir.dt.bfloat16
    Alu = mybir.AluOpType
    OT, IB, MB = cfg.OT, cfg.IB, cfg.MB

    singles = ctx.enter_context(tc.tile_pool(name=pfx + "singles", bufs=1))
    wstage = ctx.enter_context(tc.tile_pool(name=pfx + "wstage", bufs=4))
    wqpool = ctx.enter_context(tc.tile_pool(name=pfx + "wqpool", bufs=8))
    zacc = ctx.enter_context(tc.tile_pool(name=pfx + "zacc", bufs=8))
    zread = ctx.enter_context(tc.tile_pool(name=pfx + "zread", bufs=3))
    opool = ctx.enter_context(tc.tile_pool(name=pfx + "opool", bufs=4))
    small = ctx.enter_context(tc.tile_pool(name=pfx + "small", bufs=4))
    psum_pool = ctx.enter_context(tc.tile_pool(name=pfx + "psum", bufs=8, space="PSUM"))
    dram = ctx.enter_context(tc.tile_pool(name=pfx + "dram", bufs=1, space="DRAM"))

    # persistent SBUF tensors
    x_sb = [
        singles.tile([P, 2 * cfg.M_LOC], bf16, tag=f"x{j}", name=f"{pfx}x{j}")
        for j in range(IB // 2)
    ]
    stats_mb = [
        singles.tile([P, OT, 6], f32, tag=f"stats{mb}", name=f"{pfx}stats{mb}")
        for mb in range(MB)
    ]
    zkeep = [
        singles.tile([P, 2, NF], bf16, tag=f"zk{mb}", name=f"{pfx}zk{mb}")
        for mb in range(MB)
    ]
    bq_sb = singles.tile([P, cfg.OUT], bf16, tag="bq_sb")
    mu_bcast = singles.tile([P, 1], f32, tag="mu_bcast")
    z_dram = dram.tile([MB, P, 6 * NF], bf16)

    # ---- quantize b, broadcast across partitions as bf16 (emitted first:
    # ---- its Pool op is ready early, before the Pool FIFO blocks on wsl) ----
    def emit_b_path():
        BF = cfg.OUT // P
        b_pt = singles.tile([P, BF], f32, tag="b_pt", name=pfx + "b_pt")
        nc.scalar.dma_start(b_pt, b_in.rearrange("(p f) -> p f", p=P))
        bsum = small.tile([P, 1], f32, tag="bsum", name=pfx + "bsum")
        nc.vector.tensor_reduce(bsum, b_pt, axis=mybir.AxisListType.X, op=Alu.add)
        bsum_bc = small.tile([P, 1], f32, tag="bsum_bc", name=pfx + "bsum_bc")
        nc.gpsimd.partition_all_reduce(
            bsum_bc, bsum, channels=P, reduce_op=bass_isa.ReduceOp.add
        )
        bmean = small.tile([P, 1], f32, tag="bmean", name=pfx + "bmean")
        nc.scalar.mul(bmean, bsum_bc, 1.0 / float(cfg.OUT))
        bq_pt = singles.tile([P, BF], bf16, tag="bq_pt", name=pfx + "bq_pt")
        nc.vector.tensor_scalar(bq_pt, b_pt, bmean, None, op0=Alu.is_gt)
        bq_dram = dram.tile([cfg.OUT], bf16, name=pfx + "bq_dram")
        nc.scalar.dma_start(bq_dram.rearrange("(p f) -> p f", p=P), bq_pt)
        # the 1MB broadcast is only needed by the first drain (~90us in);
        # gate it off the wsl/mu critical path
        with tc.tile_wait_until(0.080):
            nc.scalar.dma_start(bq_sb, bq_dram[None, :].to_broadcast([P, cfg.OUT]))

    emit_b_path()

    # ---- global mean of W: per-core partial sum + AllReduce ----
    # wsl chunks go FIRST on the SP queue so they get HBM bandwidth before
    # the GEMM W loads (which queue behind on the same ring).
    nch = 8
    CH = cfg.WSL_F // nch  # 2048
    wm_parts = singles.tile([P, nch], f32, tag="wm_parts")
    for j in range(nch):
        wm_st = wstage.tile([P, CH], f32, tag="w_st", name=pfx + "wm_st")
        nc.sync.dma_start(wm_st, wsl[:, j * CH : (j + 1) * CH])
        nc.vector.tensor_reduce(
            wm_parts[:, j : j + 1], wm_st, axis=mybir.AxisListType.X, op=Alu.add
        )
    wm_red = small.tile([P, 1], f32, tag="wm_red")
    nc.vector.tensor_reduce(wm_red, wm_parts, axis=mybir.AxisListType.X, op=Alu.add)
    wsum_bc = small.tile([P, 1], f32, tag="wsum_bc")
    nc.gpsimd.partition_all_reduce(
        wsum_bc, wm_red, channels=P, reduce_op=bass_isa.ReduceOp.add
    )

    bounce_in = dram.tile([P, 1], f32)
    bounce_out = dram.tile([P, 1], f32)
    # bounce rides the SP ring between the wsl chunks and the W loads: FIFO
    # head-of-line blocking stops W transfers from jumping ahead of it
    nc.sync.dma_start(bounce_in[:], wsum_bc)
    if "nocoll" in flags:
        nc.gpsimd.dma_start(bounce_out[:], bounce_in[:])
    elif cfg.n_cores > 1:
        nc.gpsimd.collective_compute(
            "AllReduce",
            Alu.add,
            replica_groups=[list(range(cfg.n_cores))],
            ins=[bounce_in.opt()],
            outs=[bounce_out.opt()],
        )
    else:
        nc.gpsimd.dma_start(bounce_out[:], bounce_in[:])
    mu_raw = small.tile([P, 1], f32, tag="mu_raw")
    nc.scalar.dma_start(mu_raw, bounce_out[:])
    nc.scalar.mul(mu_bcast, mu_raw, 1.0 / float(cfg.OUT * cfg.IN))

    # ---- x loads: gpsimd cast-DMA (f32 -> bf16 in flight). Gated to start
    # ---- after the wsl read so they don't steal HBM bandwidth from the
    # ---- mu critical path (wsl -> reduce -> AllReduce).
    with tc.tile_wait_until(0.055):
        for j in range(IB // 2):
            nc.gpsimd.dma_start(x_sb[j], xT[j])

    # ---- W load + binarize: one [128, 2048] f32 DMA per (ot, ibq) ----
    w_shared = {}

    def load_w(ot, ibq, gate=None):
        if "nowdma" in flags:
            if ot not in w_shared:
                w_st = wstage.tile(
                    [P, 4 * NF], f32, tag="w_st", name=f"{pfx}w{ot}_{ibq}"
                )
                nc.sync.dma_start(w_st, Wt[ot, 0])
                w_shared[ot] = w_st
            w_st = w_shared[ot]
        else:
            w_st = wstage.tile([P, 4 * NF], f32, tag="w_st", name=f"{pfx}w{ot}_{ibq}")
            if gate is not None:
                with tc.tile_wait_until(gate):
                    nc.sync.dma_start(w_st, Wt[ot, ibq])
            else:
                nc.sync.dma_start(w_st, Wt[ot, ibq])
        wq = wqpool.tile([P, 4 * NF], bf16, tag="wq", name=f"{pfx}wq{ot}_{ibq}")
        nc.vector.tensor_scalar(wq, w_st, mu_bcast, None, op0=Alu.is_gt)
        return wq

    ddof_scale = float(cfg.OUT) / float(cfg.OUT - 1)
    zacc_live = {}
    zr_live = {}

    def drain(ot, mb, psum, zt):
        # z = psum + b_q -> bf16 in SBUF, then row stats from the bf16 copy
        par = ot % 2
        osl = slice(ot * NF, (ot + 1) * NF)
        nc.vector.tensor_tensor(zt[:, par, :], psum, bq_sb[:, osl], op=Alu.add)
        if "nonorm" in flags:
            return
        nc.vector.bn_stats(stats_mb[mb][:, ot, :], zt[:, par, :])
        if par == 1 and ot < 6:
            nc.scalar.dma_start(
                z_dram[mb, :, (ot - 1) * NF : (ot + 1) * NF],
                zt.rearrange("p a b -> p (a b)"),
            )
        if ot == 5:
            # z_dram[mb] is final after the ot5 pair-write; prefetch the
            # normalize read now so the tail isn't serialized behind it
            zr = zread.tile([P, 6 * NF], bf16, tag="zr", name=f"{pfx}zr{mb}")
            nc.sync.dma_start(zr, z_dram[mb])
            zr_live[mb] = zr

    def normalize(mb):
        mv = small.tile([P, 2], f32, tag="mv", name=f"{pfx}mv{mb}")
        nc.vector.bn_aggr(mv, stats_mb[mb])
        std = small.tile([P, 1], f32, tag="std", name=f"{pfx}std{mb}")
        nc.scalar.activation(
            std, mv[:, 1:2], mybir.ActivationFunctionType.Sqrt, scale=ddof_scale
        )
        nc.vector.tensor_scalar_add(std, std, EPS)
        rstd = small.tile([P, 1], f32, tag="rstd", name=f"{pfx}rstd{mb}")
        nc.vector.reciprocal(rstd, std)
        zr = zr_live[mb]
        for q in range(4):
            src = (
                zr[:, q * 1024 : (q + 1) * 1024]
                if q < 3
                else zkeep[mb].rearrange("p a b -> p (a b)")
            )
            o_t = opool.tile([P, 1024], f32, tag="o_t", name=f"{pfx}o{mb}_{q}")
            # split normalize TS across DVE and Pool so neither serializes
            # the tail; out DMAs alternate SP/ACT rings for the same reason
            eng = nc.vector if q < 2 else nc.gpsimd
            eng.tensor_scalar(
                o_t, src, mv[:, 0:1], rstd, op0=Alu.subtract, op1=Alu.mult
            )
            dma_eng = nc.sync if q % 2 == 0 else nc.scalar
            dma_eng.dma_start(out_t[mb, :, q * 1024 : (q + 1) * 1024], o_t)

    # ---- main GEMM ----
    for ot in range(OT):
        wqs = [load_w(ot, ibq) for ibq in range(IB // 4)]
        if ot < OT - 1:
            groups = [(0, 4), (4, 8)]
        else:
            # fully staggered: one row completes every ~7us so the 16MB of
            # normalized output streams across the whole ot7 window instead
            # of bunching after the last matmul
            groups = [(mb, mb + 1) for mb in range(MB)]
        for mlo, mhi in groups:
            psums = [
                psum_pool.tile([P, NF], f32, tag="ps", name=f"{pfx}ps{ot}_{mb}")
                for mb in range(mlo, mhi)
            ]
            ib_step = 2 if "mmhalf" in flags else (8 if "mmstep8" in flags else 1)
            for ib in range(0, IB, ib_step):
                ibq, j = ib // 4, ib % 4
                xoff = (ib % 2) * cfg.M_LOC
                for mi, mb in enumerate(range(mlo, mhi)):
                    nc.tensor.matmul(
                        psums[mi],
                        lhsT=x_sb[ib // 2][:, xoff + mb * P : xoff + mb * P + P],
                        rhs=wqs[ibq][:, j * NF : (j + 1) * NF],
                        start=(ib == 0),
                        stop=(ib == IB - ib_step),
                    )
            for mi, mb in enumerate(range(mlo, mhi)):
                if ot >= 6:
                    zt = zkeep[mb]
                else:
                    if ot % 2 == 0:
                        zt = zacc.tile(
                            [P, 2, NF], bf16, tag="za", name=f"{pfx}za{ot}_{mb}"
                        )
                        zacc_live[mb] = zt
                    else:
                        zt = zacc_live[mb]
                drain(ot, mb, psums[mi], zt)
                if ot == OT - 1 and "nonorm" not in flags:
                    normalize(mb)

    return


def build(cfg):
    import concourse.mybir as mybir
    import concourse.tile as tile
    from concourse import bacc

    f32 = mybir.dt.float32
    nc = bacc.Bacc(
        "TRN2",
        target_bir_lowering=False,
        debug=False,
        num_devices=cfg.n_cores,
    )
    xT = nc.dram_tensor(
        "xT", [cfg.IB // 2, P, 2 * cfg.M_LOC], f32, kind="ExternalInput"
    ).ap()
    Wt = nc.dram_tensor(
        "Wt", [cfg.OT, cfg.IB // 4, P, 4 * NF], f32, kind="ExternalInput"
    ).ap()
    wsl = nc.dram_tensor("wsl", [P, cfg.WSL_F], f32, kind="ExternalInput").ap()
    b_in = nc.dram_tensor("b_in", [cfg.OUT], f32, kind="ExternalInput").ap()
    out_t = nc.dram_tensor("out", [cfg.MB, P, cfg.OUT], f32, kind="ExternalOutput").ap()

    with tile.TileContext(nc) as tc:
        with ExitStack() as ctx:
            emit(ctx, tc, cfg, xT, Wt, wsl, b_in, out_t)
    nc.compile()
    return nc


def build_repeat(cfg, reps):
    """Build a variant executing the whole kernel `reps` times back-to-back
    (same I/O tensors), for slope-based device timing."""
    import concourse.mybir as mybir
    import concourse.tile as tile
    from concourse import bacc

    f32 = mybir.dt.float32
    nc = bacc.Bacc(
        "TRN2",
        target_bir_lowering=False,
        debug=False,
        num_devices=cfg.n_cores,
    )
    xT = nc.dram_tensor(
        "xT", [cfg.IB // 2, P, 2 * cfg.M_LOC], f32, kind="ExternalInput"
    ).ap()
    Wt = nc.dram_tensor(
        "Wt", [cfg.OT, cfg.IB // 4, P, 4 * NF], f32, kind="ExternalInput"
    ).ap()
    wsl = nc.dram_tensor("wsl", [P, cfg.WSL_F], f32, kind="ExternalInput").ap()
    b_in = nc.dram_tensor("b_in", [cfg.OUT], f32, kind="ExternalInput").ap()
    out_t = nc.dram_tensor("out", [cfg.MB, P, cfg.OUT], f32, kind="ExternalOutput").ap()

    with tile.TileContext(nc) as tc:
        for r in range(reps):
            with ExitStack() as ctx:
                emit(ctx, tc, cfg, xT, Wt, wsl, b_in, out_t, pfx=f"r{r}_")
    nc.compile()
    return nc


def prep_in_maps(x, W, b, cfg):
    x = np.ascontiguousarray(x, dtype=np.float32)
    W = np.ascontiguousarray(W, dtype=np.float32)
    b = np.ascontiguousarray(b, dtype=np.float32)
    # Wt[ot, ibq, p, j*NF + c] = W[ot*NF + c, (ibq*4 + j)*P + p]
    Wt = np.ascontiguousarray(
        W.reshape(cfg.OT, NF, cfg.IB // 4, 4, P).transpose(0, 2, 4, 3, 1)
    ).reshape(cfg.OT, cfg.IB // 4, P, 4 * NF)
    rows_per_core = cfg.OUT // cfg.n_cores
    in_maps = []
    for c in range(cfg.n_cores):
        xc = x[c * cfg.M_LOC : (c + 1) * cfg.M_LOC]
        # xT[ibp, p, i2*M_LOC + m] = xc[m, (ibp*2 + i2)*P + p]
        xTc = np.ascontiguousarray(
            xc.reshape(cfg.M_LOC, cfg.IB // 2, 2, P).transpose(1, 3, 2, 0)
        ).reshape(cfg.IB // 2, P, 2 * cfg.M_LOC)
        wslc = np.ascontiguousarray(
            W[c * rows_per_core : (c + 1) * rows_per_core].reshape(P, cfg.WSL_F)
        )
        in_maps.append({"xT": xTc, "Wt": Wt, "wsl": wslc, "b_in": b})
    return in_maps


class Runner:
    """Executes a compiled Bass module over 8 cores via PJRT (axon), with
    input staging separated from execution so repeated runs can be timed."""

    def __init__(self, nc, n_cores):
        import jax
        import concourse.mybir as mybir
        from concourse.bass2jax import (
            _bass_exec_p,
            install_neuronx_cc_hook,
            partition_id_tensor,
        )
        from jax.experimental.shard_map import shard_map
        from jax.sharding import Mesh, NamedSharding, PartitionSpec

        install_neuronx_cc_hook()
        self.jax = jax
        self.n_cores = n_cores
        partition_name = (
            nc.partition_id_tensor.name if nc.partition_id_tensor else None
        )
        in_names = []
        out_names = []
        out_avals = []
        self.out_shapes = []
        for alloc in nc.m.functions[0].allocations:
            if not isinstance(alloc, mybir.MemoryLocationSet):
                continue
            name = alloc.memorylocations[0].name
            if alloc.kind == "ExternalInput":
                if name != partition_name:
                    in_names.append(name)
            elif alloc.kind == "ExternalOutput":
                shape = tuple(alloc.tensor_shape)
                dtype = mybir.dt.np(alloc.dtype)
                out_names.append(name)
                out_avals.append(jax.core.ShapedArray(shape, dtype))
                self.out_shapes.append((shape, dtype))
        self.n_params = len(in_names)
        self.in_names = list(in_names)
        self.out_names = list(out_names)
        in_names_full = in_names + out_names
        if partition_name is not None:
            in_names_full.append(partition_name)

        self._p = _bass_exec_p
        self._partition_id_tensor = partition_id_tensor
        self._partition_name = partition_name
        self._bind_kwargs = dict(
            out_avals=tuple(out_avals),
            in_names=tuple(in_names_full),
            out_names=tuple(out_names),
            lowering_input_output_aliases=(),
            sim_require_finite=True,
            sim_require_nnan=True,
            nc=nc,
        )

        def _body(*args):
            operands = list(args)
            if partition_name is not None:
                operands.append(partition_id_tensor())
            outs = _bass_exec_p.bind(*operands, **self._bind_kwargs)
            return tuple(outs)

        devices = jax.devices()[:n_cores]
        assert len(devices) == n_cores
        self.mesh = Mesh(np.asarray(devices), ("core",))
        n_outs = len(out_names)
        in_specs = (PartitionSpec("core"),) * (self.n_params + n_outs)
        out_specs = (PartitionSpec("core"),) * n_outs
        self.fn = jax.jit(
            shard_map(
                _body,
                mesh=self.mesh,
                in_specs=in_specs,
                out_specs=out_specs,
                check_rep=False,
            ),
            keep_unused=True,
        )
        self.sharding = NamedSharding(self.mesh, PartitionSpec("core"))
        self.staged = None

    def stage(self, in_maps):
        jax = self.jax
        concat = [
            np.concatenate([np.asarray(m[name]) for m in in_maps], axis=0)
            for name in self.in_names
        ]
        concat += [
            np.zeros((self.n_cores * s[0], *s[1:]), d) for s, d in self.out_shapes
        ]
        self.staged = [jax.device_put(a, self.sharding) for a in concat]
        self.jax.block_until_ready(self.staged)

    def run(self):
        outs = self.fn(*self.staged)
        self.jax.block_until_ready(outs)
        return {
            name: np.asarray(outs[i]).reshape(
                self.n_cores, *self.out_shapes[i][0]
            )
            for i, name in enumerate(self.out_names)
        }


_cache = {}


def get_runner():
    cfg = FULL
    if "runner" not in _cache:
        _cache["runner"] = Runner(build(cfg), cfg.n_cores)
    return _cache["runner"]


def kernel(x, W, b):
    cfg = FULL
    assert x.shape == (cfg.M, cfg.IN) and W.shape == (cfg.OUT, cfg.IN)
    r = get_runner()
    r.stage(prep_in_maps(x, W, b, cfg))
    outs = r.run()
    out = outs["out"].reshape(cfg.n_cores * cfg.MB, P, cfg.OUT).reshape(
        cfg.M, cfg.OUT
    )
    return np.ascontiguousarray(out, dtype=np.float32)


kernel.last_exec_ns = None
